# revision 69
# baseline (speedup 1.0000x reference)
"""Trainium2 Bass kernel for nn_AlignmentEncoder.

Data-parallel over batch: 16 batches -> 8 cores x 2 batches each.

Host marshaling (make_in_maps): every tensor input is delivered bf16 in
the exact layout the kernel consumes -- keys and prior transposed, conv
weights rearranged to their SBUF layouts (qw3/qb3 pre-scaled by 2*TEMP),
biases stacked into [128, ncols] f32 columns, the valid-mask row
broadcast to [128, t2].  That removes every on-device transpose
(dma_start_transpose serializes the whole DMA stream around itself),
removes every SWDGE cast-load, and halves the prior's HBM traffic.

Per core: prior loads stream on the Pool (SWDGE) queue while both
batches' conv paths run; key path: conv k3 256->512 (PE) + relu (ACT) ->
conv k1 512->256 (PE) + bias (ACT); k2 = sum_c keT^2 (DVE square + PE
ones-reduce) -> c2row = -TEMP*k2; query path: 3-conv chain on PE with
relu/bias epilogues on ACT (qconv1/2) and DVE (qconv3, one tile per
output chunk so score tiles gate on single chunks).  Batch 0's key/query
chains are emission-interleaved; batch 1's conv work is emitted in units
interleaved into batch 0's score loop (Python generators) so no engine
queue head-of-line blocks across batches.

Scores, per 128-row t1 tile, software-pipelined with a 4-tile offset:
  phase A (tile j):  pz = 2T*qk - T*k2 (2 qk matmuls + rank-1 ones x
    c2row, one PSUM bank); logP = Ln(prior + 1e-8) (ACT, bf16);
    e1 = Exp(pz) + accum sum1 (ACT); u = prior*e1 (DVE tt, 2x);
    e2m = u*m01 + accum sum2 (DVE stt); at = e2m/sum2 (DVE recip + 4x ts)
    into the quad staging buffer.
  phase B (tile j-4): per quad one lse = Ln(sum1s) (ACT);
    lp = (pz - lse) + logP in ONE DVE stt pass (scalar operand is a
    [128,1] AP); 0.25 MB bf16 store DMAs per tile pair (SP queue).

Algebraic simplifications: the q2 term of the L2 distance cancels in
both outputs; no max-subtraction softmax is needed because z is confined
to a tiny range (TEMPERATURE = 5e-4); attn is computed in linear space,
attn = e1*prior*m01 / sum(e1*prior*m01), so the softmax over
(z + logP + M) never needs a second Exp pass and the +1e-8 inside the Ln
only matters for the logprob output.  Both outputs are stored bf16 and
upcast on the host.

Engine notes learned on this hardware: bass's first-fit activation-table
selection alternates Ln/Exp tables (1283 ns reload each); a post-compile
pass rewrites the BIR to a single load of act-table 6, which contains
ln, exp, relu, identity and copy.  DVE runs 4x only for tensor_scalar
with all-bf16 SBUF operands; accum_out or a second tensor input forces
1x.  gpsimd elementwise ops are slow Q7 software paths -- everything
elementwise lives on DVE/ACT.
"""

import numpy as np

import concourse.tile as tile
from concourse import bacc, mybir

F32 = mybir.dt.float32
BF16 = mybir.dt.bfloat16
AF = mybir.ActivationFunctionType
OP = mybir.AluOpType

B, T1, T2 = 16, 2048, 512
N_MEL, N_TEXT, N_ATT = 80, 256, 256
TEMP = 0.0005
NCORES = 8
PB = B // NCORES  # batches per core
NT1 = T1 // 128   # t1 tiles per batch
EPS = 1e-8
LAGT = 4          # score pipeline phase offset, in t1 tiles


def _dedupe_act_table_loads(nc):
    """Collapse the act-function-table loads bass inserted.

    bass's first-fit table selection maps Ln -> set 5 and Exp -> set 0, so a
    kernel alternating Ln/Exp reloads the table before nearly every
    activation (1283 ns each).  act_info.json set 6
    (natural_log_exp_and_others) contains ln, exp, relu, identity AND copy --
    every function this kernel uses -- so one load per block suffices.
    """
    for fn in nc.m.functions:
        for b in fn.blocks:
            kept_one = False
            keep = []
            for inst in b.instructions:
                if isinstance(inst, mybir.InstLoadActFuncSet):
                    if not kept_one:
                        inst.act_func_set_id = 6
                        keep.append(inst)
                        kept_one = True
                else:
                    keep.append(inst)
            b.instructions[:] = keep


def build_nc(repeat: int = 1, score_tiles: int = NT1, loop_only: bool = False):
    nc = bacc.Bacc("TRN2", target_bir_lowering=False, debug=False,
                   enable_asserts=False)

    # ---- per-core DRAM I/O ----
    # All tensor inputs arrive host-marshaled: bf16, pre-transposed /
    # pre-rearranged / pre-broadcast, weights pre-scaled where noted.  That
    # removes every SWDGE cast-load (serialized Q7 descriptor path) and
    # every on-device transpose (DMA-barrier semantics), and halves the
    # prior's HBM traffic.
    d_q = nc.dram_tensor("queries", [PB, N_MEL, T1], BF16, kind="ExternalInput").ap()
    d_k = nc.dram_tensor("keys", [PB, N_TEXT, T2], BF16, kind="ExternalInput").ap()
    d_m01 = nc.dram_tensor("m01rep", [PB, 128, T2], BF16, kind="ExternalInput").ap()
    d_pr = nc.dram_tensor("prior", [PB, T1, T2], BF16, kind="ExternalInput").ap()
    d_kw1 = nc.dram_tensor("kw1", [4, 128, 3, 2, 128], BF16, kind="ExternalInput").ap()
    d_kb1 = nc.dram_tensor("kb1", [128, 4], F32, kind="ExternalInput").ap()
    d_kw2 = nc.dram_tensor("kw2", [128, 4, N_ATT], BF16, kind="ExternalInput").ap()
    d_kb2 = nc.dram_tensor("kb2", [128, 2], F32, kind="ExternalInput").ap()
    d_qw1 = nc.dram_tensor("qw1", [N_MEL, 3, 2 * N_MEL], BF16, kind="ExternalInput").ap()
    d_qb1 = nc.dram_tensor("qb1", [128, 2], F32, kind="ExternalInput").ap()
    d_qw2a = nc.dram_tensor("qw2a", [128, N_MEL], BF16, kind="ExternalInput").ap()
    d_qw2b = nc.dram_tensor("qw2b", [32, N_MEL], BF16, kind="ExternalInput").ap()
    d_qb2 = nc.dram_tensor("qb2", [N_MEL, 1], F32, kind="ExternalInput").ap()
    d_qw3 = nc.dram_tensor("qw3", [N_MEL, N_ATT], BF16, kind="ExternalInput").ap()  # pre-scaled by 2*TEMP
    d_qb3 = nc.dram_tensor("qb3", [128, 2], F32, kind="ExternalInput").ap()  # pre-scaled by 2*TEMP
    d_attn = nc.dram_tensor("attn", [PB, 1, T1, T2], BF16, kind="ExternalOutput").ap()
    d_lp = nc.dram_tensor("attn_logprob", [PB, 1, T1, T2], BF16, kind="ExternalOutput").ap()

    with tile.TileContext(nc) as tc:
        if loop_only:
            with tc.tile_pool(name="tiny", bufs=1) as tiny:
                def ebody():
                    t = tiny.tile([128, 128], F32, tag="t", name="t")
                    nc.gpsimd.memset(t[:, 0:1], 0.0)
                    nc.sync.dma_start(out=d_attn[0, 0, 0:128, 0:128], in_=t[:])
                if repeat == 1:
                    ebody()
                else:
                    with tc.For_i(0, repeat, 1):
                        ebody()
        else:
            _body(tc, repeat, score_tiles,
                  d_q, d_k, d_m01, d_pr,
                  d_kw1, d_kb1, d_kw2, d_kb2,
                  d_qw1, d_qb1, d_qw2a, d_qw2b, d_qb2, d_qw3, d_qb3,
                  d_attn, d_lp)
    nc.compile()
    _dedupe_act_table_loads(nc)
    return nc


def _body(tc, repeat, score_tiles, d_q, d_k, d_m01, d_pr, d_kw1, d_kb1, d_kw2, d_kb2,
          d_qw1, d_qb1, d_qw2a, d_qw2b, d_qb2, d_qw3, d_qb3, d_attn, d_lp):
    nc = tc.nc
    from contextlib import ExitStack
    ctx = ExitStack()
    with ctx:
        const = ctx.enter_context(tc.tile_pool(name="const", bufs=1))
        wpool = ctx.enter_context(tc.tile_pool(name="wpool", bufs=1))
        kpool = ctx.enter_context(tc.tile_pool(name="kpool", bufs=2))
        qpool = ctx.enter_context(tc.tile_pool(name="qpool", bufs=2))
        qepool = ctx.enter_context(tc.tile_pool(name="qepool", bufs=2))
        spool = ctx.enter_context(tc.tile_pool(name="spool", bufs=3))
        lppool = ctx.enter_context(tc.tile_pool(name="lppool", bufs=8))
        smallp = ctx.enter_context(tc.tile_pool(name="smallp", bufs=3))
        sum2p = ctx.enter_context(tc.tile_pool(name="sum2p", bufs=9))
        stgpool = ctx.enter_context(tc.tile_pool(name="stgpool", bufs=3))
        prtp = ctx.enter_context(tc.tile_pool(name="prtp", bufs=8))
        ps_z = ctx.enter_context(tc.tile_pool(name="ps_z", bufs=6, space="PSUM"))
        ps_cv = ctx.enter_context(tc.tile_pool(name="ps_cv", bufs=2, space="PSUM"))

        def emit(it):
            # ---- constants ----
            ones_row = const.tile([1, 128], BF16, name=f"ones_row{it}")
            nc.vector.memset(ones_row[:], 1.0)
            ones_col = const.tile([128, 1], BF16, name=f"ones_col{it}")
            nc.vector.memset(ones_col[:], 1.0)
            eps_col = const.tile([128, 1], F32, name=f"eps_col{it}")
            nc.vector.memset(eps_col[:], EPS)

            # ---- weights: host-prepacked bf16, plain HWDGE loads on the
            # ---- ACT queue (idle this early), biases f32
            kw1_sb = [wpool.tile([128, 3, 2, 128], BF16, name=f"kw1_sb{it}_{j}")
                      for j in range(4)]
            for j in range(4):
                nc.sync.dma_start(out=kw1_sb[j][:], in_=d_kw1[j])
            kb1_sb = wpool.tile([128, 4], F32, name=f"kb1_sb{it}")
            nc.sync.dma_start(out=kb1_sb[:], in_=d_kb1)
            kw2_sb = wpool.tile([128, 4, N_ATT], BF16, name=f"kw2_sb{it}")
            nc.sync.dma_start(out=kw2_sb[:], in_=d_kw2)
            kb2_sb = wpool.tile([128, 2], F32, name=f"kb2_sb{it}")
            nc.sync.dma_start(out=kb2_sb[:], in_=d_kb2)
            qw1_sb = wpool.tile([N_MEL, 3, 2 * N_MEL], BF16, name=f"qw1_sb{it}")
            nc.sync.dma_start(out=qw1_sb[:], in_=d_qw1)
            qb1_sb = wpool.tile([128, 2], F32, name=f"qb1_sb{it}")
            nc.sync.dma_start(out=qb1_sb[:], in_=d_qb1)
            qw2a_sb = wpool.tile([128, N_MEL], BF16, name=f"qw2a_sb{it}")
            nc.sync.dma_start(out=qw2a_sb[:], in_=d_qw2a)
            qw2b_sb = wpool.tile([32, N_MEL], BF16, name=f"qw2b_sb{it}")
            nc.sync.dma_start(out=qw2b_sb[:], in_=d_qw2b)
            qb2_sb = wpool.tile([N_MEL, 1], F32, name=f"qb2_sb{it}")
            nc.sync.dma_start(out=qb2_sb[:], in_=d_qb2)
            qw3_sb = wpool.tile([N_MEL, N_ATT], BF16, name=f"qw3_sb{it}")
            nc.sync.dma_start(out=qw3_sb[:], in_=d_qw3)
            qb3_sb = wpool.tile([128, 2], F32, name=f"qb3_sb{it}")
            nc.sync.dma_start(out=qb3_sb[:], in_=d_qb3)

            ST = score_tiles
            pend = []      # (j, pz, logP, e2m, sum2, sum1s)
            aq = {}        # phase-A quad state (sum1s tile)
            bq = {}        # phase-B quad state (lses, lp4, at4, store args)

            def phase_a(j, i, qeT, keT, c2row, prT, m01rep):
                k4 = j % 4
                if k4 == 0:
                    aq['sum1s'] = smallp.tile([128, 4], F32, tag="sum1s",
                                              name="sum1s")
                    aq['at4'] = stgpool.tile([128, 4, T2], BF16, tag="at4",
                                             name="at4")
                sum1s = aq['sum1s']
                at4 = aq['at4']
                pz = ps_z.tile([128, T2], F32, tag="pz", name="pz")
                c0 = (i % 4) * 128
                nc.tensor.matmul(pz[:], qeT[0][i // 4][:, c0:c0 + 128],
                                 keT[0][:], start=True, stop=False)
                nc.tensor.matmul(pz[:], qeT[1][i // 4][:, c0:c0 + 128],
                                 keT[1][:], start=False, stop=False)
                nc.tensor.matmul(pz[:], ones_row[:], c2row[:],
                                 start=False, stop=True)
                prv = prT[i // 4][:, i % 4, :]
                logP_t = lppool.tile([128, T2], BF16, tag="logP", name="logP")
                nc.scalar.activation(logP_t[:], prv, AF.Ln, bias=eps_col[:])
                logP = logP_t[:]
                e1 = spool.tile([128, T2], BF16, tag="e1", name="e1")
                nc.scalar.activation(e1[:], pz[:], AF.Exp,
                                     accum_out=sum1s[:, k4:k4 + 1])
                u = spool.tile([128, T2], BF16, tag="u", name="u")
                nc.vector.tensor_mul(u[:], prv, e1[:])
                e2m = lppool.tile([128, T2], BF16, tag="e2m", name="e2m")
                sum2 = sum2p.tile([128, 1], F32, tag="sum2", name="sum2")
                nc.vector.scalar_tensor_tensor(
                    e2m[:], u[:], 1.0, m01rep[:],
                    OP.mult, OP.mult, accum_out=sum2[:])
                r2 = sum2p.tile([128, 1], F32, tag="r2", name="r2")
                nc.vector.reciprocal(r2[:], sum2[:])
                nc.vector.tensor_scalar(at4[:, k4, :], e2m[:], r2[:],
                                        None, OP.mult)
                return (j, pz, logP, at4, sum1s)

            def phase_b(entry):
                j, pz, logP, at4, sum1s = entry
                k4 = j % 4
                if k4 == 0:
                    lses = smallp.tile([128, 4], F32, tag="lses", name="lses")
                    nc.scalar.activation(lses[:], sum1s[:], AF.Ln)
                    bq['lses'] = lses
                    bq['lp4'] = stgpool.tile([128, 4, T2], BF16, tag="lp4",
                                             name="lp4")
                lses, lp4 = bq['lses'], bq['lp4']
                # lp = (z - lse) + logP in one DVE pass (scalar is [128,1] AP)
                nc.vector.scalar_tensor_tensor(
                    lp4[:, k4, :], pz[:], lses[:, k4:k4 + 1], logP,
                    OP.subtract, OP.add)
                if k4 % 2 == 1:
                    b, i0 = divmod(j - 1, ST)
                    h = k4 // 2
                    nc.sync.dma_start(
                        out=d_lp[b, 0, i0 * 128:(i0 + 2) * 128, :]
                        .rearrange("(g p) t -> p g t", p=128),
                        in_=lp4[:, 2 * h:2 * h + 2, :])
                    nc.sync.dma_start(
                        out=d_attn[b, 0, i0 * 128:(i0 + 2) * 128, :]
                        .rearrange("(g p) t -> p g t", p=128),
                        in_=at4[:, 2 * h:2 * h + 2, :])

            # ===== input loads for both batches, before the prior chain:
            # every dma_start_transpose acts as a DMA barrier, so anything
            # emitted after one stalls behind the whole prior chain.
            keysT_all, qT_all, m01rep_all, prT_all = [], [], [], []
            for b in range(PB):
                keysT = [kpool.tile([128, T2 + 2], BF16, tag=f"keysT{ci}",
                                    name=f"keysT{ci}") for ci in range(2)]
                for ci in range(2):
                    nc.vector.memset(keysT[ci][:, 0:1], 0.0)
                    nc.vector.memset(keysT[ci][:, T2 + 1:T2 + 2], 0.0)
                    nc.gpsimd.dma_start(
                        out=keysT[ci][:, 1:T2 + 1],
                        in_=d_k[b, ci * 128:(ci + 1) * 128, :])
                keysT_all.append(keysT)
                qT = qpool.tile([N_MEL, T1 + 2], BF16, tag="qT")
                nc.vector.memset(qT[:, 0:1], 0.0)
                nc.vector.memset(qT[:, T1 + 1:T1 + 2], 0.0)
                nc.gpsimd.dma_start(out=qT[:, 1:T1 + 1], in_=d_q[b])
                qT_all.append(qT)
                m01rep = kpool.tile([128, T2], BF16, tag="m01rep")
                nc.sync.dma_start(out=m01rep[:], in_=d_m01[b])
                m01rep_all.append(m01rep)
                # prior loads (bf16 [t1, t2] from the host) queued per
                # batch so batch 0's prior precedes batch 1's inputs
                quads = []
                for q in range(NT1 // 4):
                    prq = prtp.tile([128, 4, T2], BF16, tag="prq", name="prq")
                    nc.gpsimd.dma_start(
                        out=prq[:],
                        in_=d_pr[b, q * 512:(q + 1) * 512, :]
                        .rearrange("(g p) t -> p g t", p=128))
                    quads.append(prq)
                prT_all.append(quads)

            kprod = []
            qprod = []

            def key_units(b):
                # ================= key path =================
                keysT = keysT_all[b]
                # kconv1 (k=3, 256->512) + relu
                ke1T = [kpool.tile([128, T2], BF16, tag=f"ke1T{jj}", name=f"ke1T{jj}") for jj in range(4)]
                for jj in range(4):
                    pcv = ps_cv.tile([128, T2], F32, tag="pcv")
                    first = True
                    for dt in range(3):
                        for ci in range(2):
                            nc.tensor.matmul(
                                pcv[:], kw1_sb[jj][:, dt, ci, :],
                                keysT[ci][:, dt:dt + T2],
                                start=first, stop=(dt == 2 and ci == 1))
                            first = False
                    nc.scalar.activation(ke1T[jj][:], pcv[:], AF.Relu,
                                         bias=kb1_sb[:, jj:jj + 1])
                    yield
                # kconv2 (k=1, 512->256)
                keT = [kpool.tile([128, T2], BF16, tag=f"keT{j2}", name=f"keT{j2}") for j2 in range(2)]
                for j2 in range(2):
                    pcv = ps_cv.tile([128, T2], F32, tag="pcv")
                    for ci1 in range(4):
                        nc.tensor.matmul(pcv[:], kw2_sb[:, ci1, j2 * 128:(j2 + 1) * 128],
                                         ke1T[ci1][:],
                                         start=(ci1 == 0), stop=(ci1 == 3))
                    nc.vector.tensor_scalar(keT[j2][:], pcv[:],
                                            kb2_sb[:, j2:j2 + 1], None, OP.add)
                    yield
                # k2 = sum_c keT^2 ; c2row = -TEMP * k2
                sqk = [kpool.tile([128, T2], BF16, tag=f"sqk{j2}", name=f"sqk{j2}") for j2 in range(2)]
                for j2 in range(2):
                    nc.vector.tensor_mul(sqk[j2][:], keT[j2][:], keT[j2][:])
                pk2 = ps_cv.tile([1, T2], F32, tag="pcv", name="pk2")
                for j2 in range(2):
                    nc.tensor.matmul(pk2[:], ones_col[:], sqk[j2][:],
                                     start=(j2 == 0), stop=(j2 == 1))
                c2row = kpool.tile([1, T2], BF16, tag="c2row")
                nc.scalar.activation(c2row[:], pk2[:], AF.Copy, scale=-TEMP)

                kprod.append((keT, c2row, m01rep_all[b]))
                yield

            def query_units(b):
                # ================= query path =================
                qT = qT_all[b]
                # qconv1 (k=3, 80->160) + relu: o-tiles [128, 32]
                qe1a = qpool.tile([128, T1], BF16, tag="qe1a")
                qe1b = qpool.tile([32, T1], BF16, tag="qe1b")
                for n in range(4):
                    for (oi, (qe1, o0, ow)) in enumerate(
                            [(qe1a, 0, 128), (qe1b, 128, 32)]):
                        pcv = ps_cv.tile([128, T2], F32, tag="pcv")
                        for dt in range(3):
                            nc.tensor.matmul(
                                pcv[0:ow, :], qw1_sb[:, dt, o0:o0 + ow],
                                qT[:, dt + n * T2:dt + (n + 1) * T2],
                                start=(dt == 0), stop=(dt == 2))
                        nc.scalar.activation(
                            qe1[:, n * T2:(n + 1) * T2], pcv[0:ow, :],
                            AF.Relu, bias=qb1_sb[0:ow, oi:oi + 1])
                        yield
                # qconv2 (k=1, 160->80) + relu
                qe2 = qpool.tile([N_MEL, T1], BF16, tag="qe2")
                for n in range(4):
                    pcv = ps_cv.tile([128, T2], F32, tag="pcv")
                    nc.tensor.matmul(pcv[0:N_MEL, :], qw2a_sb[:],
                                     qe1a[:, n * T2:(n + 1) * T2],
                                     start=True, stop=False)
                    nc.tensor.matmul(pcv[0:N_MEL, :], qw2b_sb[:],
                                     qe1b[:, n * T2:(n + 1) * T2],
                                     start=False, stop=True)
                    if n == 0:
                        nc.vector.tensor_scalar(qe2[:, n * T2:(n + 1) * T2],
                                                pcv[0:N_MEL, :], qb2_sb[:],
                                                0.0, OP.add, OP.max)
                    else:
                        nc.scalar.activation(qe2[:, n * T2:(n + 1) * T2],
                                             pcv[0:N_MEL, :], AF.Relu,
                                             bias=qb2_sb[:])
                    yield
                # qconv3 (k=1, 80->256), scaled by 2*TEMP; one tile per
                # (o, n) chunk so score tiles gate on single chunks
                qeT = [[qepool.tile([128, T2], BF16, tag=f"qeT{o}_{n}",
                                    name=f"qeT{o}_{n}") for n in range(4)]
                       for o in range(2)]
                for n in range(4):
                    for o in range(2):
                        pcv = ps_cv.tile([128, T2], F32, tag="pcv")
                        nc.tensor.matmul(pcv[:], qw3_sb[:, o * 128:(o + 1) * 128],
                                         qe2[:, n * T2:(n + 1) * T2],
                                         start=True, stop=True)
                        if n == 0:
                            nc.scalar.activation(qeT[o][n][:], pcv[:],
                                                 AF.Identity,
                                                 bias=qb3_sb[:, o:o + 1])
                        else:
                            nc.vector.tensor_scalar(qeT[o][n][:], pcv[:],
                                                    qb3_sb[:, o:o + 1],
                                                    None, OP.add)
                        yield
                qprod.append(qeT)

            def conv_units(b):
                yield from key_units(b)
                yield from query_units(b)

            # ================= scores =================
            # batch 0's key and query conv chains are independent --
            # interleave them so the PE/ACT ping-pong of one fills the
            # other's bubbles; batch 1's conv units are interleaved into
            # batch 0's score loop so no engine queue head-of-line blocks
            # on the other batch's dependencies.
            assert ST % 4 == 0
            kg, qg = key_units(0), query_units(0)
            alive = [kg, qg]
            while alive:
                for g in list(alive):
                    if next(g, StopIteration) is StopIteration:
                        alive.remove(g)
            g1 = conv_units(1)
            for i in range(ST):
                if len(pend) >= LAGT:
                    phase_b(pend.pop(0))
                keT, c2row, m01rep = kprod[0]
                pend.append(phase_a(i, i, qprod[0], keT, c2row,
                                    prT_all[0], m01rep))
                next(g1, None)
                next(g1, None)
            for _ in g1:
                pass
            for i in range(ST):
                if len(pend) >= LAGT:
                    phase_b(pend.pop(0))
                keT, c2row, m01rep = kprod[1]
                pend.append(phase_a(ST + i, i, qprod[1], keT, c2row,
                                    prT_all[1], m01rep))
            while pend:
                phase_b(pend.pop(0))

        if repeat == 1:
            emit(0)
        else:
            with tc.For_i(0, repeat, 1):
                emit(0)


_CACHE = {}


def _get_nc(repeat: int = 1, score_tiles: int = NT1, loop_only: bool = False):
    key = (repeat, score_tiles, loop_only)
    if key not in _CACHE:
        _CACHE[key] = build_nc(repeat, score_tiles, loop_only)
    return _CACHE[key]


def make_in_maps(queries, keys, mask, attn_prior,
                 kw1, kb1, kw2, kb2, qw1, qb1, qw2, qb2, qw3, qb3):
    import ml_dtypes
    BF = ml_dtypes.bfloat16

    def bf(x):
        return np.ascontiguousarray(np.asarray(x, dtype=np.float32).astype(BF))

    def f32(x):
        return np.ascontiguousarray(x, dtype=np.float32)

    queries = bf(queries)
    keysT = bf(np.asarray(keys, dtype=np.float32).transpose(0, 2, 1))
    priorT = bf(np.asarray(attn_prior, dtype=np.float32).transpose(0, 2, 1))
    m01 = (1.0 - np.asarray(mask, dtype=np.float32)).astype(BF)
    m01rep = np.ascontiguousarray(
        np.broadcast_to(m01[:, None, :], (B, 128, m01.shape[-1])))

    # weight prepack: the exact SBUF layouts the kernel consumes
    kw1p = bf(np.asarray(kw1, dtype=np.float32)
              .reshape(3, 2, 128, 4, 128).transpose(3, 2, 0, 1, 4))
    kw2p = bf(np.asarray(kw2, dtype=np.float32)
              .reshape(2 * N_TEXT, N_ATT).reshape(4, 128, N_ATT)
              .transpose(1, 0, 2))
    qw1p = bf(np.asarray(qw1, dtype=np.float32).transpose(1, 0, 2))
    qw2f = np.asarray(qw2, dtype=np.float32).reshape(2 * N_MEL, N_MEL)
    qw3p = bf(np.asarray(qw3, dtype=np.float32).reshape(N_MEL, N_ATT)
              * (2.0 * TEMP))
    kb1p = f32(np.asarray(kb1, dtype=np.float32).reshape(4, 128).T)
    kb2p = f32(np.asarray(kb2, dtype=np.float32).reshape(2, 128).T)
    qb1p = np.zeros((128, 2), np.float32)
    qb1p[0:128, 0] = np.asarray(qb1, dtype=np.float32)[0:128]
    qb1p[0:32, 1] = np.asarray(qb1, dtype=np.float32)[128:160]
    qb2p = f32(np.asarray(qb2, dtype=np.float32).reshape(N_MEL, 1))
    qb3p = f32(np.asarray(qb3, dtype=np.float32).reshape(2, 128).T
               * (2.0 * TEMP))
    w = dict(kw1=kw1p, kb1=kb1p, kw2=kw2p, kb2=kb2p,
             qw1=qw1p, qb1=qb1p, qw2a=bf(qw2f[0:128]), qw2b=bf(qw2f[128:160]),
             qb2=qb2p, qw3=qw3p, qb3=qb3p)
    in_maps = []
    for c in range(NCORES):
        s = slice(c * PB, (c + 1) * PB)
        in_maps.append(dict(
            queries=queries[s], keys=keysT[s], m01rep=m01rep[s], prior=priorT[s],
            **w))
    return in_maps


def kernel(queries, keys, mask, attn_prior,
           kw1, kb1, kw2, kb2, qw1, qb1, qw2, qb2, qw3, qb3):
    from concourse import bass_utils
    nc = _get_nc(1)
    in_maps = make_in_maps(queries, keys, mask, attn_prior,
                           kw1, kb1, kw2, kb2, qw1, qb1, qw2, qb2, qw3, qb3)
    res = bass_utils.run_bass_kernel_spmd(nc, in_maps, core_ids=list(range(NCORES)))
    attn = np.concatenate([res.results[c]["attn"].astype(np.float32)
                           for c in range(NCORES)], axis=0)
    lp = np.concatenate([res.results[c]["attn_logprob"].astype(np.float32)
                         for c in range(NCORES)], axis=0)
    return attn, lp
